# revision 34
# baseline (speedup 1.0000x reference)
"""DeepCross kernel for 8x TRN2 NeuronCores.

Math: the cross-network keeps temp = x0 * f with f a per-row scalar, so the
whole model collapses to G = x0 @ [cross_w | w1 | wf_x0]  ([B, 37]) plus a
tiny per-row tail:
    g = G[:, :4]; p1 = G[:, 4:36]; q = G[:, 36:37]
    f1 = 1 + g0 + b0; f2 = f1*g1 + b1; f3 = f2*(1+g2) + b2; f4 = f3*g3 + b3
    h1 = relu(f4 * p1); h2 = relu(h1 @ w2); out = sigmoid(h2 @ wf_h + q + bf)

Device strategy (data-parallel over batch, 1024 rows/core):
  - emb table quad-packed to fp8 e4m3 [25000, 256B]; dma_gather with
    idx = x//4 (fits int16) in slot-major order.
  - 4 SWDGE queues, one gather per queue per subtile: descriptor drain
    runs on 4 rings concurrently (~2 ns/desc aggregate; fewer active
    rings contend on the ring partitions' AXI ports and run ~2x slower).
  - 1-of-4 select via 3 predicated copies on int32 bitcast views
    (whole-row moves, 4x fewer DVE elements), one ACT cast fp8->bf16.
  - PE-transpose 128x128 chunks, accumulate G^T [37, 128] per subtile.
  - Tail computed incrementally per subtile so it hides under the
    remaining gather waves; one [1, 1024] f32 row DMA'd out per core.
"""
import sys
sys.path.insert(0, '/opt/trn_rl_repo')
import os
import numpy as np
import ml_dtypes

from concourse import bass, mybir
import concourse.tile as tile
from concourse import bacc, library_config
from concourse.bass_utils import run_bass_kernel_spmd
from concourse.masks import make_identity
from concourse.tile import add_dep_helper

BF16 = ml_dtypes.bfloat16

B, T, E = 8192, 128, 64
V = 100000
D = T * E                 # 8192
L = 4
H1, H2 = 32, 16
NCORES = 8
BC = B // NCORES          # 1024 batch rows per core
NSUB = BC // 128          # 8 subtiles of 128 rows
GS = 32                   # slots (trees) per gather group
NQ = 4                    # SWDGE queues (Q7 core pairs)
NGRP = NSUB * NQ          # 32 gather groups per core
NIDX = 128 * GS           # 4096 indices per gather
QE = 256                  # fp8 elements per quad row (256 B)
UQ = V // 4               # 25000 quad rows
NCHUNK = D // 128         # 64 d-chunks per subtile
NW = L + H1 + 1           # 37 fused weight columns
IDXF = NQ * (NIDX // 16)  # idx free-dim words per subtile

_PROGRAM = None
KMODE = os.environ.get('KMODE', 'full')


def _build_program():
    f32 = mybir.dt.float32
    bf16 = mybir.dt.bfloat16
    fp8 = mybir.dt.float8e4
    nc = bacc.Bacc("TRN2", target_bir_lowering=False, debug=False,
                   num_devices=NCORES, dynamic_dma_scratch_size=32768,
                   num_swdge_queues=NQ)

    tblq = nc.dram_tensor("tblq", [UQ, QE], fp8, kind="ExternalInput")
    xidx = nc.dram_tensor("xidx", [128, NSUB * IDXF], mybir.dt.int16,
                          kind="ExternalInput")
    xmask = nc.dram_tensor("xmask", [128, NGRP * 3 * GS], mybir.dt.uint8,
                           kind="ExternalInput")
    wbd = nc.dram_tensor("wb", [128, NCHUNK * NW], bf16, kind="ExternalInput")
    w2d = nc.dram_tensor("w2", [H1, H2], f32, kind="ExternalInput")
    wfhd = nc.dram_tensor("wfh", [H2, 1], f32, kind="ExternalInput")
    cbd = nc.dram_tensor("cb", [1, L], f32, kind="ExternalInput")
    b1d = nc.dram_tensor("b1v", [1, H1], f32, kind="ExternalInput")
    b2d = nc.dram_tensor("b2v", [1, H2], f32, kind="ExternalInput")
    bfd = nc.dram_tensor("bfv", [1, 1], f32, kind="ExternalInput")
    outd = nc.dram_tensor("out", [NSUB, 128], f32, kind="ExternalOutput")

    AF = mybir.ActivationFunctionType
    OP = mybir.AluOpType

    with tile.TileContext(nc) as tc:
        with (
            tc.tile_pool(name="const", bufs=1) as cpool,
            tc.tile_pool(name="quad", bufs=8) as qpool,
            tc.tile_pool(name="x0c", bufs=4) as xpool,
            tc.tile_pool(name="xt", bufs=4) as xtpool,
            tc.tile_pool(name="tail", bufs=2) as tpool,
            tc.tile_pool(name="ptp", bufs=4, space="PSUM") as ptpool,
            tc.tile_pool(name="pgt", bufs=2, space="PSUM") as pgpool,
            tc.tile_pool(name="pts", bufs=2, space="PSUM") as pspool,
        ):
            nc.gpsimd.load_library(library_config.mlp)

            wb_t = cpool.tile([128, NCHUNK * NW], bf16)
            nc.sync.dma_start(out=wb_t[:], in_=wbd.ap())
            ident = cpool.tile([128, 128], bf16)
            make_identity(nc, ident[:])
            ident32 = cpool.tile([128, 128], f32)
            make_identity(nc, ident32[:])
            w2_t = cpool.tile([H1, H2], f32)
            nc.sync.dma_start(out=w2_t[:], in_=w2d.ap())
            wfh_t = cpool.tile([H2, 1], f32)
            nc.sync.dma_start(out=wfh_t[:], in_=wfhd.ap())
            # pack [cb(4) | b1(32) | b2(16) | bf(1)] into one row, broadcast
            # to all 128 partitions via a k=1 matmul with a ones column.
            NPK = L + H1 + H2 + 1
            pack_t = cpool.tile([1, NPK], f32)
            nc.sync.dma_start(out=pack_t[0:1, 0:L], in_=cbd.ap())
            nc.sync.dma_start(out=pack_t[0:1, L:L + H1], in_=b1d.ap())
            nc.sync.dma_start(out=pack_t[0:1, L + H1:L + H1 + H2], in_=b2d.ap())
            nc.sync.dma_start(out=pack_t[0:1, L + H1 + H2:NPK], in_=bfd.ap())
            ones_r = cpool.tile([1, 128], f32)
            nc.vector.memset(ones_r[:], 1.0)
            packb_p = pspool.tile([128, NPK], f32, tag="tps")
            packb_mm = nc.tensor.matmul(out=packb_p[:], lhsT=ones_r[:],
                                        rhs=pack_t[:], start=True, stop=True)
            prev_tail_pe = packb_mm.ins
            packb = cpool.tile([128, NPK], f32)
            nc.vector.tensor_copy(out=packb[:], in_=packb_p[:])
            cbb = packb[:, 0:L]
            b1b = packb[:, L:L + H1]
            b2b = packb[:, L + H1:L + H1 + H2]
            bfb = packb[:, L + H1 + H2:NPK]
            out_col = cpool.tile([128, NSUB], f32, tag="out_col")
            idx_all = cpool.tile([128, NSUB * IDXF], mybir.dt.int16,
                                 tag="idx_all")
            # split the idx load so the first gather starts early
            nc.sync.dma_start(out=idx_all[:, 0:IDXF],
                              in_=xidx.ap()[:, 0:IDXF])
            nc.sync.dma_start(out=idx_all[:, IDXF:],
                              in_=xidx.ap()[:, IDXF:])
            msk_all = cpool.tile([128, NGRP * 3 * GS], mybir.dt.uint8,
                                 tag="msk_all")
            nc.sync.dma_start(out=msk_all[:], in_=xmask.ap())

            for sub in range(NSUB):
                gt = pgpool.tile([NW, 128], f32, tag="gt")
                for qh in range(NQ):
                    g = sub * NQ + qh
                    idx_t = idx_all[:, g * (NIDX // 16):(g + 1) * (NIDX // 16)]
                    msk_t = msk_all[:, g * 3 * GS:(g + 1) * 3 * GS]

                    quad = qpool.tile([128, GS * QE], fp8, tag="quad")
                    qview = quad[:].rearrange("p (s e) -> p s e", e=QE)
                    nc.gpsimd.dma_gather(
                        out_ap=qview[:, :, :],
                        in_ap=tblq.ap(),
                        idxs_ap=idx_t[:, :],
                        num_idxs=NIDX,
                        num_idxs_reg=NIDX,
                        elem_size=QE,
                        single_packet=False,
                        queue_num=qh,
                    )

                    x0c8 = xpool.tile([128, GS * 64], fp8, tag="x0c8")
                    x0c = xpool.tile([128, GS * 64], bf16, tag="x0c")
                    # The select moves whole 64 B rows; run the predicated
                    # copies on int32 bitcast views (4x fewer DVE elements).
                    # The init copy stays on real fp8 views: ACT copies pass
                    # values through the float datapath, so a bitcast there
                    # would mangle the bytes.
                    qv = quad[:].rearrange("p (s e) -> p s e", e=QE)
                    xv = x0c8[:].rearrange("p (s e) -> p s e", e=64)
                    qv32 = quad[:].bitcast(mybir.dt.int32).rearrange(
                        "p (s e) -> p s e", e=QE // 4)
                    xv32 = x0c8[:].bitcast(mybir.dt.int32).rearrange(
                        "p (s e) -> p s e", e=16)

                    def _mk(i):
                        m = msk_t[:, i * GS:(i + 1) * GS]
                        m = m.rearrange("p (s one) -> p s one", one=1)
                        return m.to_broadcast([128, GS, 16])

                    # 1-of-4 select: init with row 0, overwrite rows 1..3
                    # under host-built equality masks (r==1, r==2, r==3),
                    # then one cast pass fp8 -> bf16 for the PE.
                    nc.scalar.copy(out=xv[:, :, :], in_=qv[:, :, 0:64])
                    for r in range(1, 4):
                        nc.vector.copy_predicated(
                            out=xv32[:, :, :], mask=_mk(r - 1),
                            data=qv32[:, :, r * 16:(r + 1) * 16])
                    nc.scalar.copy(out=x0c[:], in_=x0c8[:])
                    for c8 in range(GS // 16):   # 2 batches of 8 chunks
                        tp = ptpool.tile([128, 1024], bf16, tag="tp")
                        for j in range(8):
                            c2 = c8 * 8 + j
                            nc.tensor.transpose(
                                out=tp[:, j * 128:(j + 1) * 128],
                                in_=x0c[:, c2 * 128:(c2 + 1) * 128],
                                identity=ident[:],
                            )
                        xt = xtpool.tile([128, 1024], bf16, tag="xt")
                        nc.vector.tensor_copy(
                            out=xt[:].bitcast(mybir.dt.int32),
                            in_=tp[:].bitcast(mybir.dt.int32))
                        for j in range(8):
                            cd = qh * (GS // 2) + c8 * 8 + j
                            mm = nc.tensor.matmul(
                                out=gt[:],
                                lhsT=wb_t[:, cd * NW:(cd + 1) * NW],
                                rhs=xt[:, j * 128:(j + 1) * 128],
                                start=(cd == 0),
                                stop=(cd == NCHUNK - 1),
                            )
                            if cd == 0 and sub == 0 and prev_tail_pe is not None:
                                add_dep_helper(mm.ins, prev_tail_pe,
                                               reason="packb before accum groups")

                # ---- incremental per-subtile tail (hides under gathers) ----
                gs0 = tpool.tile([NW, 128], f32, tag="gs0")
                nc.scalar.copy(out=gs0[:], in_=gt[:])
                gtt_p = pspool.tile([128, 64], f32, tag="tps")
                nc.tensor.transpose(out=gtt_p[:, 0:NW], in_=gs0[:],
                                    identity=ident32[0:NW, 0:NW])
                gs_t = tpool.tile([128, NW], f32, tag="gs_t")
                nc.scalar.copy(out=gs_t[:], in_=gtt_p[:, 0:NW])

                # f-recurrence on [128, 1] columns
                fv = tpool.tile([128, 4], f32, tag="fv")
                nc.vector.tensor_scalar(out=fv[:, 0:1], in0=gs_t[:, 0:1],
                                        scalar1=cbb[:, 0:1], scalar2=1.0,
                                        op0=OP.add, op1=OP.add)
                nc.vector.tensor_tensor(out=fv[:, 1:2], in0=fv[:, 0:1],
                                        in1=gs_t[:, 1:2], op=OP.mult)
                nc.vector.tensor_scalar(out=fv[:, 1:2], in0=fv[:, 1:2],
                                        scalar1=cbb[:, 1:2], scalar2=None,
                                        op0=OP.add)
                nc.vector.tensor_scalar(out=fv[:, 2:3], in0=gs_t[:, 2:3],
                                        scalar1=1.0, scalar2=None, op0=OP.add)
                nc.vector.tensor_tensor(out=fv[:, 2:3], in0=fv[:, 1:2],
                                        in1=fv[:, 2:3], op=OP.mult)
                nc.vector.tensor_scalar(out=fv[:, 2:3], in0=fv[:, 2:3],
                                        scalar1=cbb[:, 2:3], scalar2=None,
                                        op0=OP.add)
                nc.vector.tensor_tensor(out=fv[:, 3:4], in0=fv[:, 2:3],
                                        in1=gs_t[:, 3:4], op=OP.mult)
                nc.vector.tensor_scalar(out=fv[:, 3:4], in0=fv[:, 3:4],
                                        scalar1=cbb[:, 3:4], scalar2=None,
                                        op0=OP.add)

                # h1 = relu(f4 * p1 + b1)  [128, H1]
                h1_t = tpool.tile([128, H1], f32, tag="h1_t")
                nc.vector.tensor_tensor(
                    out=h1_t[:], in0=gs_t[:, L:L + H1],
                    in1=fv[:, 3:4].to_broadcast([128, H1]), op=OP.mult)
                nc.vector.tensor_tensor(out=h1_t[:], in0=h1_t[:], in1=b1b,
                                        op=OP.add)
                nc.scalar.activation(out=h1_t[:], in_=h1_t[:], func=AF.Relu)

                # h2 = relu(h1 @ w2 + b2)
                h1T_p = pspool.tile([128, 128], f32, tag="tps")
                nc.tensor.transpose(out=h1T_p[0:H1, :], in_=h1_t[:],
                                    identity=ident32[:])
                h1T = tpool.tile([H1, 128], f32, tag="h1T")
                nc.scalar.copy(out=h1T[:], in_=h1T_p[0:H1, :])
                h2p = pspool.tile([128, H2], f32, tag="tps")
                nc.tensor.matmul(out=h2p[:], lhsT=h1T[:], rhs=w2_t[:],
                                 start=True, stop=True)
                h2_t = tpool.tile([128, H2], f32, tag="h2_t")
                nc.vector.tensor_tensor(out=h2_t[:], in0=h2p[:], in1=b2b,
                                        op=OP.add)
                nc.scalar.activation(out=h2_t[:], in_=h2_t[:], func=AF.Relu)

                # z = h2 @ wf_h ; out_col[:, sub] = sigmoid(z + q + bf)
                h2T_p = pspool.tile([128, 128], f32, tag="tps")
                nc.tensor.transpose(out=h2T_p[0:H2, :], in_=h2_t[:],
                                    identity=ident32[:])
                h2T = tpool.tile([H2, 128], f32, tag="h2T")
                nc.scalar.copy(out=h2T[:], in_=h2T_p[0:H2, :])
                zp = pspool.tile([128, 1], f32, tag="tps")
                nc.tensor.matmul(out=zp[:], lhsT=h2T[:], rhs=wfh_t[:],
                                 start=True, stop=True)
                z_t = tpool.tile([128, 1], f32, tag="z_t")
                nc.vector.tensor_tensor(out=z_t[:], in0=zp[:],
                                        in1=gs_t[:, NW - 1:NW], op=OP.add)
                nc.scalar.activation(out=out_col[:, sub:sub + 1], in_=z_t[:],
                                     func=AF.Sigmoid,
                                     bias=bfb[:, 0:1], scale=1.0)

            oT_p = pspool.tile([NSUB, 128], f32, tag="tps")
            nc.tensor.transpose(out=oT_p[:], in_=out_col[:],
                                identity=ident32[:])
            oT = cpool.tile([NSUB, 128], f32)
            nc.scalar.copy(out=oT[:], in_=oT_p[:])
            nc.sync.dma_start(out=outd.ap(), in_=oT[:])

    nc.compile()
    return nc


def _get_program():
    global _PROGRAM
    if _PROGRAM is None:
        _PROGRAM = _build_program()
    return _PROGRAM


def _host_prep(x, emb, cross_w, cross_b, w1, b1, w2, b2, wf, bf):
    x = np.asarray(x)
    emb = np.ascontiguousarray(np.asarray(emb, dtype=np.float32))
    cross_w = np.asarray(cross_w, dtype=np.float32)
    cross_b = np.asarray(cross_b, dtype=np.float32)
    w1 = np.asarray(w1, dtype=np.float32)
    w2 = np.asarray(w2, dtype=np.float32)
    b1 = np.asarray(b1, dtype=np.float32)
    b2 = np.asarray(b2, dtype=np.float32)
    wf = np.asarray(wf, dtype=np.float32)
    bf = np.asarray(bf, dtype=np.float32)

    tblq = emb.astype(ml_dtypes.float8_e4m3fn).reshape(UQ, QE)
    wbig = np.concatenate([cross_w[:, :, 0].T, w1, wf[H2:, :]], axis=1)  # [D, 37]
    wb_np = np.ascontiguousarray(
        wbig.reshape(NCHUNK, 128, NW).transpose(1, 0, 2).reshape(128, NCHUNK * NW)
    ).astype(BF16)

    shared = {
        "tblq": tblq,
        "wb": wb_np,
        "w2": w2,
        "wfh": np.ascontiguousarray(wf[:H2, :]),
        "cb": cross_b.reshape(1, L),
        "b1v": b1.reshape(1, H1),
        "b2v": b2.reshape(1, H2),
        "bfv": bf.reshape(1, 1),
    }

    in_maps = []
    for c in range(NCORES):
        xc = x[c * BC:(c + 1) * BC].astype(np.int64)
        xq = (xc // 4).astype(np.int16)          # [1024, 128]
        xr = (xc % 4).astype(np.int8)
        idx_np = np.empty((NGRP, 128, NIDX // 16), dtype=np.int16)
        msk_np = np.empty((NGRP, 128, 3 * GS), dtype=np.uint8)
        for g in range(NGRP):
            s, qh = g // NQ, g % NQ
            blk = xq[s * 128:(s + 1) * 128, qh * GS:(qh + 1) * GS]  # [128b, GSt]
            lst = blk.T.reshape(-1)                                 # i = t*128+b
            idx_np[g] = np.tile(lst.reshape(NIDX // 16, 16).T, (8, 1))
            rb = xr[s * 128:(s + 1) * 128, qh * GS:(qh + 1) * GS]
            msk_np[g, :, 0 * GS:1 * GS] = (rb == 1).astype(np.uint8)
            msk_np[g, :, 1 * GS:2 * GS] = (rb == 2).astype(np.uint8)
            msk_np[g, :, 2 * GS:3 * GS] = (rb == 3).astype(np.uint8)
        m = dict(shared)
        m["xidx"] = np.ascontiguousarray(idx_np.transpose(1, 0, 2).reshape(128, -1))
        m["xmask"] = np.ascontiguousarray(msk_np.transpose(1, 0, 2).reshape(128, -1))
        in_maps.append(m)
    return in_maps


def _ensure_ntff_hook():
    """The image's antenv lacks axon_hooks; synthesize it so
    run_bass_kernel_spmd(trace=True) can NTFF-profile via the axon .so."""
    import types
    if 'antenv.axon_hooks' in sys.modules:
        return
    import antenv
    mod = types.ModuleType('antenv.axon_hooks')
    _state = {'hook': None}
    def set_axon_ntff_profile_hook(h):
        _state['hook'] = h
    def get_axon_ntff_profile_hook():
        if _state['hook'] is None:
            try:
                from trn_agent_boot.trn_boot import _ntff_profile_via_ctypes
                _state['hook'] = _ntff_profile_via_ctypes('/opt/axon/libaxon_pjrt.so')
            except Exception:
                return None
        return _state['hook']
    mod.set_axon_ntff_profile_hook = set_axon_ntff_profile_hook
    mod.get_axon_ntff_profile_hook = get_axon_ntff_profile_hook
    sys.modules['antenv.axon_hooks'] = mod
    antenv.axon_hooks = mod


def run(inputs: dict, trace: bool = False):
    if trace:
        _ensure_ntff_hook()
    nc = _get_program()
    in_maps = _host_prep(**inputs)
    res = run_bass_kernel_spmd(nc, in_maps, core_ids=list(range(NCORES)),
                               trace=trace)
    out = np.concatenate(
        [np.asarray(res.results[c]["out"]).reshape(BC, 1) for c in range(NCORES)]
    )
    return out.astype(np.float32), res


def kernel(**inputs):
    out, _ = run(inputs, trace=False)
    return out


# revision 35
# speedup vs baseline: 1.0359x; 1.0359x over previous
"""DeepCross kernel for 8x TRN2 NeuronCores.

Math: the cross-network keeps temp = x0 * f with f a per-row scalar, so the
whole model collapses to G = x0 @ [cross_w | w1 | wf_x0]  ([B, 37]) plus a
tiny per-row tail:
    g = G[:, :4]; p1 = G[:, 4:36]; q = G[:, 36:37]
    f1 = 1 + g0 + b0; f2 = f1*g1 + b1; f3 = f2*(1+g2) + b2; f4 = f3*g3 + b3
    h1 = relu(f4 * p1); h2 = relu(h1 @ w2); out = sigmoid(h2 @ wf_h + q + bf)

Device strategy (data-parallel over batch, 1024 rows/core):
  - emb table quad-packed to fp8 e4m3 [25000, 256B]; dma_gather with
    idx = x//4 (fits int16) in slot-major order.
  - 4 SWDGE queues, one gather per queue per subtile: descriptor drain
    runs on 4 rings concurrently (~2 ns/desc aggregate; fewer active
    rings contend on the ring partitions' AXI ports and run ~2x slower).
  - 1-of-4 select via 3 predicated copies on int32 bitcast views
    (whole-row moves, 4x fewer DVE elements), one ACT cast fp8->bf16.
  - PE-transpose 128x128 chunks, accumulate G^T [37, 128] per subtile.
  - Tail computed incrementally per subtile so it hides under the
    remaining gather waves; one [1, 1024] f32 row DMA'd out per core.
"""
import sys
sys.path.insert(0, '/opt/trn_rl_repo')
import os
import numpy as np
import ml_dtypes

from concourse import bass, mybir
import concourse.tile as tile
from concourse import bacc, library_config
from concourse.bass_utils import run_bass_kernel_spmd
from concourse.masks import make_identity
from concourse.tile import add_dep_helper

BF16 = ml_dtypes.bfloat16

B, T, E = 8192, 128, 64
V = 100000
D = T * E                 # 8192
L = 4
H1, H2 = 32, 16
NCORES = 8
BC = B // NCORES          # 1024 batch rows per core
NSUB = BC // 128          # 8 subtiles of 128 rows
GS = 16                   # slots (trees) per gather group
NQ = 4                    # SWDGE queues (Q7 core pairs)
NGRP = NSUB * 2 * NQ      # 64 gather groups per core
NIDX = 128 * GS           # 4096 indices per gather
QE = 256                  # fp8 elements per quad row (256 B)
UQ = V // 4               # 25000 quad rows
NCHUNK = D // 128         # 64 d-chunks per subtile
NW = L + H1 + 1           # 37 fused weight columns
IDXF = 2 * NQ * (NIDX // 16)  # idx free-dim words per subtile

_PROGRAM = None
KMODE = os.environ.get('KMODE', 'full')


def _build_program():
    f32 = mybir.dt.float32
    bf16 = mybir.dt.bfloat16
    fp8 = mybir.dt.float8e4
    nc = bacc.Bacc("TRN2", target_bir_lowering=False, debug=False,
                   num_devices=NCORES, dynamic_dma_scratch_size=32768,
                   num_swdge_queues=NQ)

    tblq = nc.dram_tensor("tblq", [UQ, QE], fp8, kind="ExternalInput")
    xidx = nc.dram_tensor("xidx", [128, NSUB * IDXF], mybir.dt.int16,
                          kind="ExternalInput")
    xmask = nc.dram_tensor("xmask", [128, NGRP * 3 * GS], mybir.dt.uint8,
                           kind="ExternalInput")
    wbd = nc.dram_tensor("wb", [128, NCHUNK * NW], bf16, kind="ExternalInput")
    w2d = nc.dram_tensor("w2", [H1, H2], f32, kind="ExternalInput")
    wfhd = nc.dram_tensor("wfh", [H2, 1], f32, kind="ExternalInput")
    cbd = nc.dram_tensor("cb", [1, L], f32, kind="ExternalInput")
    b1d = nc.dram_tensor("b1v", [1, H1], f32, kind="ExternalInput")
    b2d = nc.dram_tensor("b2v", [1, H2], f32, kind="ExternalInput")
    bfd = nc.dram_tensor("bfv", [1, 1], f32, kind="ExternalInput")
    outd = nc.dram_tensor("out", [NSUB, 128], f32, kind="ExternalOutput")

    AF = mybir.ActivationFunctionType
    OP = mybir.AluOpType

    with tile.TileContext(nc) as tc:
        with (
            tc.tile_pool(name="const", bufs=1) as cpool,
            tc.tile_pool(name="quad", bufs=8) as qpool,
            tc.tile_pool(name="x0c", bufs=4) as xpool,
            tc.tile_pool(name="xt", bufs=4) as xtpool,
            tc.tile_pool(name="tail", bufs=2) as tpool,
            tc.tile_pool(name="ptp", bufs=4, space="PSUM") as ptpool,
            tc.tile_pool(name="pgt", bufs=2, space="PSUM") as pgpool,
            tc.tile_pool(name="pts", bufs=2, space="PSUM") as pspool,
        ):
            nc.gpsimd.load_library(library_config.mlp)

            wb_t = cpool.tile([128, NCHUNK * NW], bf16)
            nc.sync.dma_start(out=wb_t[:], in_=wbd.ap())
            ident = cpool.tile([128, 128], bf16)
            make_identity(nc, ident[:])
            ident32 = cpool.tile([128, 128], f32)
            make_identity(nc, ident32[:])
            w2_t = cpool.tile([H1, H2], f32)
            nc.sync.dma_start(out=w2_t[:], in_=w2d.ap())
            wfh_t = cpool.tile([H2, 1], f32)
            nc.sync.dma_start(out=wfh_t[:], in_=wfhd.ap())
            # pack [cb(4) | b1(32) | b2(16) | bf(1)] into one row, broadcast
            # to all 128 partitions via a k=1 matmul with a ones column.
            NPK = L + H1 + H2 + 1
            pack_t = cpool.tile([1, NPK], f32)
            nc.sync.dma_start(out=pack_t[0:1, 0:L], in_=cbd.ap())
            nc.sync.dma_start(out=pack_t[0:1, L:L + H1], in_=b1d.ap())
            nc.sync.dma_start(out=pack_t[0:1, L + H1:L + H1 + H2], in_=b2d.ap())
            nc.sync.dma_start(out=pack_t[0:1, L + H1 + H2:NPK], in_=bfd.ap())
            ones_r = cpool.tile([1, 128], f32)
            nc.vector.memset(ones_r[:], 1.0)
            packb_p = pspool.tile([128, NPK], f32, tag="tps")
            packb_mm = nc.tensor.matmul(out=packb_p[:], lhsT=ones_r[:],
                                        rhs=pack_t[:], start=True, stop=True)
            prev_tail_pe = packb_mm.ins
            packb = cpool.tile([128, NPK], f32)
            nc.vector.tensor_copy(out=packb[:], in_=packb_p[:])
            cbb = packb[:, 0:L]
            b1b = packb[:, L:L + H1]
            b2b = packb[:, L + H1:L + H1 + H2]
            bfb = packb[:, L + H1 + H2:NPK]
            out_col = cpool.tile([128, NSUB], f32, tag="out_col")
            idx_all = cpool.tile([128, NSUB * IDXF], mybir.dt.int16,
                                 tag="idx_all")
            # split the idx load so the first gather starts early
            nc.sync.dma_start(out=idx_all[:, 0:IDXF],
                              in_=xidx.ap()[:, 0:IDXF])
            nc.sync.dma_start(out=idx_all[:, IDXF:],
                              in_=xidx.ap()[:, IDXF:])
            msk_all = cpool.tile([128, NGRP * 3 * GS], mybir.dt.uint8,
                                 tag="msk_all")
            nc.sync.dma_start(out=msk_all[:], in_=xmask.ap())

            for sub in range(NSUB):
                gt = pgpool.tile([NW, 128], f32, tag="gt")
                for g2 in range(2 * NQ):
                    qh = g2 % NQ
                    g = sub * 2 * NQ + g2
                    idx_t = idx_all[:, g * (NIDX // 16):(g + 1) * (NIDX // 16)]
                    msk_t = msk_all[:, g * 3 * GS:(g + 1) * 3 * GS]

                    quad = qpool.tile([128, GS * QE], fp8, tag="quad")
                    qview = quad[:].rearrange("p (s e) -> p s e", e=QE)
                    nc.gpsimd.dma_gather(
                        out_ap=qview[:, :, :],
                        in_ap=tblq.ap(),
                        idxs_ap=idx_t[:, :],
                        num_idxs=NIDX,
                        num_idxs_reg=NIDX,
                        elem_size=QE,
                        single_packet=False,
                        queue_num=qh,
                    )

                    x0c8 = xpool.tile([128, GS * 64], fp8, tag="x0c8")
                    x0c = xpool.tile([128, GS * 64], bf16, tag="x0c")
                    # The select moves whole 64 B rows; run the predicated
                    # copies on int32 bitcast views (4x fewer DVE elements).
                    # The init copy stays on real fp8 views: ACT copies pass
                    # values through the float datapath, so a bitcast there
                    # would mangle the bytes.
                    qv = quad[:].rearrange("p (s e) -> p s e", e=QE)
                    xv = x0c8[:].rearrange("p (s e) -> p s e", e=64)
                    qv32 = quad[:].bitcast(mybir.dt.int32).rearrange(
                        "p (s e) -> p s e", e=QE // 4)
                    xv32 = x0c8[:].bitcast(mybir.dt.int32).rearrange(
                        "p (s e) -> p s e", e=16)

                    def _mk(i):
                        m = msk_t[:, i * GS:(i + 1) * GS]
                        m = m.rearrange("p (s one) -> p s one", one=1)
                        return m.to_broadcast([128, GS, 16])

                    # 1-of-4 select: init with row 0, overwrite rows 1..3
                    # under host-built equality masks (r==1, r==2, r==3),
                    # then one cast pass fp8 -> bf16 for the PE.
                    nc.scalar.copy(out=xv[:, :, :], in_=qv[:, :, 0:64])
                    for r in range(1, 4):
                        nc.vector.copy_predicated(
                            out=xv32[:, :, :], mask=_mk(r - 1),
                            data=qv32[:, :, r * 16:(r + 1) * 16])
                    nc.scalar.copy(out=x0c[:], in_=x0c8[:])
                    for c8 in range(GS // 16):   # 2 batches of 8 chunks
                        tp = ptpool.tile([128, 1024], bf16, tag="tp")
                        for j in range(8):
                            c2 = c8 * 8 + j
                            nc.tensor.transpose(
                                out=tp[:, j * 128:(j + 1) * 128],
                                in_=x0c[:, c2 * 128:(c2 + 1) * 128],
                                identity=ident[:],
                            )
                        xt = xtpool.tile([128, 1024], bf16, tag="xt")
                        nc.vector.tensor_copy(
                            out=xt[:].bitcast(mybir.dt.int32),
                            in_=tp[:].bitcast(mybir.dt.int32))
                        for j in range(8):
                            cd = g2 * (GS // 2) + c8 * 8 + j
                            mm = nc.tensor.matmul(
                                out=gt[:],
                                lhsT=wb_t[:, cd * NW:(cd + 1) * NW],
                                rhs=xt[:, j * 128:(j + 1) * 128],
                                start=(cd == 0),
                                stop=(cd == NCHUNK - 1),
                            )
                            if cd == 0 and sub == 0 and prev_tail_pe is not None:
                                add_dep_helper(mm.ins, prev_tail_pe,
                                               reason="packb before accum groups")

                # ---- incremental per-subtile tail (hides under gathers) ----
                gs0 = tpool.tile([NW, 128], f32, tag="gs0")
                nc.scalar.copy(out=gs0[:], in_=gt[:])
                gtt_p = pspool.tile([128, 64], f32, tag="tps")
                nc.tensor.transpose(out=gtt_p[:, 0:NW], in_=gs0[:],
                                    identity=ident32[0:NW, 0:NW])
                gs_t = tpool.tile([128, NW], f32, tag="gs_t")
                nc.scalar.copy(out=gs_t[:], in_=gtt_p[:, 0:NW])

                # f-recurrence on [128, 1] columns
                fv = tpool.tile([128, 4], f32, tag="fv")
                nc.vector.tensor_scalar(out=fv[:, 0:1], in0=gs_t[:, 0:1],
                                        scalar1=cbb[:, 0:1], scalar2=1.0,
                                        op0=OP.add, op1=OP.add)
                nc.vector.tensor_tensor(out=fv[:, 1:2], in0=fv[:, 0:1],
                                        in1=gs_t[:, 1:2], op=OP.mult)
                nc.vector.tensor_scalar(out=fv[:, 1:2], in0=fv[:, 1:2],
                                        scalar1=cbb[:, 1:2], scalar2=None,
                                        op0=OP.add)
                nc.vector.tensor_scalar(out=fv[:, 2:3], in0=gs_t[:, 2:3],
                                        scalar1=1.0, scalar2=None, op0=OP.add)
                nc.vector.tensor_tensor(out=fv[:, 2:3], in0=fv[:, 1:2],
                                        in1=fv[:, 2:3], op=OP.mult)
                nc.vector.tensor_scalar(out=fv[:, 2:3], in0=fv[:, 2:3],
                                        scalar1=cbb[:, 2:3], scalar2=None,
                                        op0=OP.add)
                nc.vector.tensor_tensor(out=fv[:, 3:4], in0=fv[:, 2:3],
                                        in1=gs_t[:, 3:4], op=OP.mult)
                nc.vector.tensor_scalar(out=fv[:, 3:4], in0=fv[:, 3:4],
                                        scalar1=cbb[:, 3:4], scalar2=None,
                                        op0=OP.add)

                # h1 = relu(f4 * p1 + b1)  [128, H1]
                h1_t = tpool.tile([128, H1], f32, tag="h1_t")
                nc.vector.tensor_tensor(
                    out=h1_t[:], in0=gs_t[:, L:L + H1],
                    in1=fv[:, 3:4].to_broadcast([128, H1]), op=OP.mult)
                nc.vector.tensor_tensor(out=h1_t[:], in0=h1_t[:], in1=b1b,
                                        op=OP.add)
                nc.scalar.activation(out=h1_t[:], in_=h1_t[:], func=AF.Relu)

                # h2 = relu(h1 @ w2 + b2)
                h1T_p = pspool.tile([128, 128], f32, tag="tps")
                nc.tensor.transpose(out=h1T_p[0:H1, :], in_=h1_t[:],
                                    identity=ident32[:])
                h1T = tpool.tile([H1, 128], f32, tag="h1T")
                nc.scalar.copy(out=h1T[:], in_=h1T_p[0:H1, :])
                h2p = pspool.tile([128, H2], f32, tag="tps")
                nc.tensor.matmul(out=h2p[:], lhsT=h1T[:], rhs=w2_t[:],
                                 start=True, stop=True)
                h2_t = tpool.tile([128, H2], f32, tag="h2_t")
                nc.vector.tensor_tensor(out=h2_t[:], in0=h2p[:], in1=b2b,
                                        op=OP.add)
                nc.scalar.activation(out=h2_t[:], in_=h2_t[:], func=AF.Relu)

                # z = h2 @ wf_h ; out_col[:, sub] = sigmoid(z + q + bf)
                h2T_p = pspool.tile([128, 128], f32, tag="tps")
                nc.tensor.transpose(out=h2T_p[0:H2, :], in_=h2_t[:],
                                    identity=ident32[:])
                h2T = tpool.tile([H2, 128], f32, tag="h2T")
                nc.scalar.copy(out=h2T[:], in_=h2T_p[0:H2, :])
                zp = pspool.tile([128, 1], f32, tag="tps")
                nc.tensor.matmul(out=zp[:], lhsT=h2T[:], rhs=wfh_t[:],
                                 start=True, stop=True)
                z_t = tpool.tile([128, 1], f32, tag="z_t")
                nc.vector.tensor_tensor(out=z_t[:], in0=zp[:],
                                        in1=gs_t[:, NW - 1:NW], op=OP.add)
                nc.scalar.activation(out=out_col[:, sub:sub + 1], in_=z_t[:],
                                     func=AF.Sigmoid,
                                     bias=bfb[:, 0:1], scale=1.0)

            oT_p = pspool.tile([NSUB, 128], f32, tag="tps")
            nc.tensor.transpose(out=oT_p[:], in_=out_col[:],
                                identity=ident32[:])
            oT = cpool.tile([NSUB, 128], f32)
            nc.scalar.copy(out=oT[:], in_=oT_p[:])
            nc.sync.dma_start(out=outd.ap(), in_=oT[:])

    nc.compile()
    return nc


def _get_program():
    global _PROGRAM
    if _PROGRAM is None:
        _PROGRAM = _build_program()
    return _PROGRAM


def _host_prep(x, emb, cross_w, cross_b, w1, b1, w2, b2, wf, bf):
    x = np.asarray(x)
    emb = np.ascontiguousarray(np.asarray(emb, dtype=np.float32))
    cross_w = np.asarray(cross_w, dtype=np.float32)
    cross_b = np.asarray(cross_b, dtype=np.float32)
    w1 = np.asarray(w1, dtype=np.float32)
    w2 = np.asarray(w2, dtype=np.float32)
    b1 = np.asarray(b1, dtype=np.float32)
    b2 = np.asarray(b2, dtype=np.float32)
    wf = np.asarray(wf, dtype=np.float32)
    bf = np.asarray(bf, dtype=np.float32)

    tblq = emb.astype(ml_dtypes.float8_e4m3fn).reshape(UQ, QE)
    wbig = np.concatenate([cross_w[:, :, 0].T, w1, wf[H2:, :]], axis=1)  # [D, 37]
    wb_np = np.ascontiguousarray(
        wbig.reshape(NCHUNK, 128, NW).transpose(1, 0, 2).reshape(128, NCHUNK * NW)
    ).astype(BF16)

    shared = {
        "tblq": tblq,
        "wb": wb_np,
        "w2": w2,
        "wfh": np.ascontiguousarray(wf[:H2, :]),
        "cb": cross_b.reshape(1, L),
        "b1v": b1.reshape(1, H1),
        "b2v": b2.reshape(1, H2),
        "bfv": bf.reshape(1, 1),
    }

    in_maps = []
    for c in range(NCORES):
        xc = x[c * BC:(c + 1) * BC].astype(np.int64)
        xq = (xc // 4).astype(np.int16)          # [1024, 128]
        xr = (xc % 4).astype(np.int8)
        idx_np = np.empty((NGRP, 128, NIDX // 16), dtype=np.int16)
        msk_np = np.empty((NGRP, 128, 3 * GS), dtype=np.uint8)
        for g in range(NGRP):
            s, g2 = g // (2 * NQ), g % (2 * NQ)
            blk = xq[s * 128:(s + 1) * 128, g2 * GS:(g2 + 1) * GS]  # [128b, GSt]
            lst = blk.T.reshape(-1)                                 # i = t*128+b
            idx_np[g] = np.tile(lst.reshape(NIDX // 16, 16).T, (8, 1))
            rb = xr[s * 128:(s + 1) * 128, g2 * GS:(g2 + 1) * GS]
            msk_np[g, :, 0 * GS:1 * GS] = (rb == 1).astype(np.uint8)
            msk_np[g, :, 1 * GS:2 * GS] = (rb == 2).astype(np.uint8)
            msk_np[g, :, 2 * GS:3 * GS] = (rb == 3).astype(np.uint8)
        m = dict(shared)
        m["xidx"] = np.ascontiguousarray(idx_np.transpose(1, 0, 2).reshape(128, -1))
        m["xmask"] = np.ascontiguousarray(msk_np.transpose(1, 0, 2).reshape(128, -1))
        in_maps.append(m)
    return in_maps


def _ensure_ntff_hook():
    """The image's antenv lacks axon_hooks; synthesize it so
    run_bass_kernel_spmd(trace=True) can NTFF-profile via the axon .so."""
    import types
    if 'antenv.axon_hooks' in sys.modules:
        return
    import antenv
    mod = types.ModuleType('antenv.axon_hooks')
    _state = {'hook': None}
    def set_axon_ntff_profile_hook(h):
        _state['hook'] = h
    def get_axon_ntff_profile_hook():
        if _state['hook'] is None:
            try:
                from trn_agent_boot.trn_boot import _ntff_profile_via_ctypes
                _state['hook'] = _ntff_profile_via_ctypes('/opt/axon/libaxon_pjrt.so')
            except Exception:
                return None
        return _state['hook']
    mod.set_axon_ntff_profile_hook = set_axon_ntff_profile_hook
    mod.get_axon_ntff_profile_hook = get_axon_ntff_profile_hook
    sys.modules['antenv.axon_hooks'] = mod
    antenv.axon_hooks = mod


def run(inputs: dict, trace: bool = False):
    if trace:
        _ensure_ntff_hook()
    nc = _get_program()
    in_maps = _host_prep(**inputs)
    res = run_bass_kernel_spmd(nc, in_maps, core_ids=list(range(NCORES)),
                               trace=trace)
    out = np.concatenate(
        [np.asarray(res.results[c]["out"]).reshape(BC, 1) for c in range(NCORES)]
    )
    return out.astype(np.float32), res


def kernel(**inputs):
    out, _ = run(inputs, trace=False)
    return out
